# revision 6
# baseline (speedup 1.0000x reference)
"""TRN2 Bass kernel for nn_CrossLayerAttention: head-parallel tensor-parallel
over 8 NeuronCores.

v3: PE-continuity-focused schedule. The TRN2 PE clock ramps (0.65/1.2/2.4 GHz)
with sustained execution and back-to-back matmuls issue every ~216ns, so the
kernel is organized to keep the tensor-engine queue free of dependency waits:

  - host pre-permutes h.T into per-sweep strip layout: one 1 MB DMA per
    projection sweep (no per-chunk DMA issue pressure on the Sync queue)
  - per sweep: all K matmuls (fp8 DoubleRow, 256-deep), then all V matmuls
    (natural layout, strip-stationary), then the rms ones-matmul into a PSUM
    slot recycled from the drained K accumulator (kps/ssq share a 2-ring)
  - rmsnorm chain entirely off-PE: DVE drain + square, PE ones-matmul, Scalar
    sqrt, DVE reciprocal_approx_accurate, DVE scale
  - attention: QK bf16; exp over [128,1024] PSUM pairs -> fp8 e pairs; AV
    (fp8 DoubleRow) emitted TWO chunks behind QK so the PE never waits on the
    exp stream; Z accumulated bf16 on DVE + ones-matmul
  - out_proj bf16, PSUM->bf16 drains on DVE; partials bf16; 8 ReduceScatter
    chunks overlapped with attention; dummy warmup collective at t=0;
    epilogues scheduled >=3 chunks behind their RS
  - SIREN positional field computed on host (input-independent), added in the
    epilogue before the final rmsnorm (scale-invariance folds the fp8 scales)
"""
import numpy as np
import ml_dtypes
from contextlib import ExitStack

import concourse.bass as bass
import concourse.tile as tile
from concourse import bacc, mybir
from concourse.bass_utils import run_bass_kernel_spmd

P = 128
L = 2048
C = 2048
H = 16
D = 128
NCORES = 8
HPC = H // NCORES          # heads per core
CL = HPC * D               # local channels per core
LKV = 2 * L                # kv length (2 history entries)
EPS = 1e-6
NQB = 4                    # attention q blocks (512 wide)
NCH = 8                    # out_proj / RS chunks (256 rows)
SH = L // 8 // NCH         # shard rows per RS chunk (32)
WS = 32.0                  # wq/wk fp8 scale (cancels in q/k rmsnorm)
VS = 16.0                  # wv fp8 scale (cancels in final rmsnorm)

f32 = mybir.dt.float32
bf16 = mybir.dt.bfloat16
f8 = mybir.dt.float8e4
FT = mybir.ActivationFunctionType
OP = mybir.AluOpType
PM = mybir.MatmulPerfMode
BF = ml_dtypes.bfloat16
F8 = ml_dtypes.float8_e4m3

_CACHE = {}


def _build_program():
    nc = bacc.Bacc("TRN2", target_bir_lowering=False, debug=False,
                   num_devices=NCORES)

    # ---- DRAM I/O ----
    # hTp[t]: pre-permuted strips, [p, (q4, cc, j, col)]
    hTp = [nc.dram_tensor(f"hTp{t}", [P, 4 * 8192], f8, kind="ExternalInput")
           for t in range(3)]
    wq = nc.dram_tensor("wq", [P, 4096], f8, kind="ExternalInput")
    wk0 = nc.dram_tensor("wk0", [P, 4096], f8, kind="ExternalInput")
    wk1 = nc.dram_tensor("wk1", [P, 4096], f8, kind="ExternalInput")
    wv = nc.dram_tensor("wv", [P, 4096], f8, kind="ExternalInput")
    wo = nc.dram_tensor("wo", [CL, C], bf16, kind="ExternalInput")
    onw = nc.dram_tensor("onw", [P, C], f32, kind="ExternalInput")
    xs = nc.dram_tensor("xs", [NCH * SH, C], f32, kind="ExternalInput")
    pos = nc.dram_tensor("pos", [NCH * SH, C], bf16, kind="ExternalInput")
    out = nc.dram_tensor("o", [NCH * SH, C], f32, kind="ExternalOutput")

    partial = [nc.dram_tensor(f"partial{k}", [256, C], bf16)
               for k in range(NCH)]
    rs_out = [nc.dram_tensor(f"rs_out{k}", [SH, C], bf16) for k in range(NCH)]
    wdum = nc.dram_tensor("wdum", [8, 8], f32)
    wrs = nc.dram_tensor("wrs", [1, 8], f32)

    with tile.TileContext(nc) as tc, ExitStack() as ctx:
        const = ctx.enter_context(tc.tile_pool(name="const", bufs=1))
        persist = ctx.enter_context(tc.tile_pool(name="persist", bufs=1))

        # ---- constants ----
        ones_t = const.tile([P, P], f32)
        nc.vector.memset(ones_t[:], 1.0)
        ones_b = const.tile([P, P], bf16)
        nc.vector.tensor_copy(ones_b[:], ones_t[:])
        eps_t = const.tile([P, 1], f32)
        nc.vector.memset(eps_t[:], EPS)

        # ---- warmup collective (absorb ring setup during projections) ----
        wdum_sb = const.tile([8, 8], f32)
        nc.vector.memset(wdum_sb[:], 0.0)
        nc.sync.dma_start(wdum[:], wdum_sb[:])
        nc.gpsimd.collective_compute(
            "ReduceScatter", OP.add,
            replica_groups=[list(range(NCORES))],
            ins=[wdum[:]], outs=[wrs[:]],
        )

        # ---- persistent activations / weights ----
        QTa = persist.tile([P, HPC, L], bf16, name="QTa")
        KTa = persist.tile([P, HPC, LKV], bf16, name="KTa")
        Va = persist.tile([P, LKV // 256, 2, CL], f8, name="Va")
        OTa = persist.tile([P, HPC, L], bf16, name="OTa")
        wo_sb = persist.tile([P, HPC, C], bf16, name="wo_sb")
        onw_sb = persist.tile([P, C], f32, name="onw_sb")

        # ================= projections =================
        wp_cm = tc.tile_pool(name="wp", bufs=1)
        wp = wp_cm.__enter__()
        wq_sb = wp.tile([P, 8, 2, HPC, D], f8, name="wq_sb")
        wk_sb = [wp.tile([P, 8, 2, HPC, D], f8, name=f"wk{t}_sb")
                 for t in range(2)]
        wv_sb = wp.tile([P, 8, 2, CL], f8, name="wv_sb")
        # weight DMAs needed by the first sweeps go first
        nc.sync.dma_start(
            wk_sb[0][:].rearrange("p a j h m -> p (a j h m)"), wk0[:])
        nc.sync.dma_start(wv_sb[:].rearrange("p a j m -> p (a j m)"), wv[:])

        pp_cm = tc.tile_pool(name="pp", bufs=1, space="PSUM")
        pp = pp_cm.__enter__()
        hp_cm = tc.tile_pool(name="hp", bufs=2)
        hp = hp_cm.__enter__()
        rp_cm = tc.tile_pool(name="rp", bufs=2)
        rp = rp_cm.__enter__()

        def sweep(t, q4, w_sb, dst_tile, dst_off, with_v):
            """Project strips of hT[t], cols [512*q4, +512): K (+V), 2 heads."""
            strip = hp.tile([P, 8, 2, 512], f8, name="strip", tag="strip")
            nc.sync.dma_start(
                strip[:].rearrange("p a j q -> p (a j q)"),
                hTp[t][:, q4 * 8192:(q4 + 1) * 8192])
            kps = pp.tile([P, 1024], f32, name="kps", tag="kps", bufs=2)
            # K: all 16 matmuls back-to-back (strip prefetched, no deps)
            for cc in range(8):
                for h in range(HPC):
                    nc.tensor.matmul(
                        kps[:, h * 512:(h + 1) * 512],
                        w_sb[:, cc, :, h, :], strip[:, cc, :, :],
                        start=(cc == 0), stop=(cc == 7),
                        perf_mode=PM.DoubleRow)
            # rms drain chain starts (off-PE) while V matmuls run
            raw = rp.tile([P, 1024], bf16, name="raw", tag="raw")
            nc.vector.tensor_copy(raw[:], kps[:])
            sq = rp.tile([P, 1024], bf16, name="sq", tag="sq")
            nc.vector.tensor_mul(sq[:], raw[:], raw[:])
            if with_v:
                vps = [pp.tile([P, 256], f32, name=f"vb{lb}", tag=f"vb{lb}",
                               bufs=1) for lb in range(4)]
                for cc in range(8):
                    for lb in range(4):
                        nc.tensor.matmul(
                            vps[lb][:],
                            strip[:, cc, :, lb * P:(lb + 1) * P],
                            wv_sb[:, cc, :, :],
                            start=(cc == 0), stop=(cc == 7),
                            perf_mode=PM.DoubleRow)
                for lb in range(4):
                    ck = t * 16 + q4 * 4 + lb
                    nc.vector.tensor_copy(Va[:, ck // 2, ck % 2, :],
                                          vps[lb][:])
            # partition-sum of squares into the PSUM slot the K drain freed
            ssq = pp.tile([P, 1024], f32, name="ssq", tag="kps", bufs=2)
            for half in range(2):
                nc.tensor.matmul(ssq[:, half * 512:(half + 1) * 512],
                                 ones_b[:], sq[:, half * 512:(half + 1) * 512],
                                 start=True, stop=True)
            rms = rp.tile([P, 1024], f32, name="rms", tag="rms")
            nc.scalar.activation(rms[:], ssq[:], FT.Sqrt,
                                 bias=eps_t[:, 0:1], scale=1.0 / D)
            inv = rp.tile([P, 1024], f32, name="inv", tag="inv")
            scr8 = rp.tile([P, 1024], f32, name="scr8", tag="scr8")
            nc.vector.reciprocal_approx_accurate(inv[:], rms[:], scr8[:])
            nc.vector.tensor_mul(
                dst_tile[:, :, dst_off:dst_off + 512],
                raw[:].rearrange("p (h q) -> p h q", h=2),
                inv[:].rearrange("p (h q) -> p h q", h=2))

        sweep(0, 0, wk_sb[0], KTa, 0, True)
        # remaining weight loads, issued behind the first sweep's strip DMA
        nc.sync.dma_start(
            wk_sb[1][:].rearrange("p a j h m -> p (a j h m)"), wk1[:])
        nc.sync.dma_start(wq_sb[:].rearrange("p a j h m -> p (a j h m)"),
                          wq[:])
        for h in range(HPC):
            nc.sync.dma_start(wo_sb[:, h, :], wo[h * P:(h + 1) * P, :])
        nc.sync.dma_start(onw_sb[:], onw[:])
        for q4 in range(1, 4):
            sweep(0, q4, wk_sb[0], KTa, q4 * 512, True)
        for q4 in range(4):
            sweep(1, q4, wk_sb[1], KTa, L + q4 * 512, True)
        for q4 in range(4):
            sweep(2, q4, wq_sb, QTa, q4 * 512, False)

        rp_cm.__exit__(None, None, None)
        hp_cm.__exit__(None, None, None)
        pp_cm.__exit__(None, None, None)
        wp_cm.__exit__(None, None, None)

        # ===== attention / out_proj / RS / epilogue, PE-order interleaved ====
        with (tc.tile_pool(name="pssp", bufs=2, space="PSUM") as pssp,
              tc.tile_pool(name="pozp", bufs=2, space="PSUM") as pozp,
              tc.tile_pool(name="pbp", bufs=2, space="PSUM") as pbp,
              tc.tile_pool(name="ep", bufs=5) as ep,
              tc.tile_pool(name="zp", bufs=2) as zp,
              tc.tile_pool(name="ob", bufs=3) as ob,
              tc.tile_pool(name="epi", bufs=2) as epi):

            def att(qb, h):
                po = pozp.tile([P, 512], f32, name="po", tag="poz")
                zacc = zp.tile([P, 1024], bf16, name="zacc", tag="zacc")
                q_ap = QTa[:, h, qb * 512:(qb + 1) * 512]
                pend = []  # AV emission lags QK by 2 chunks: PE never waits
                for cc in range(16):
                    pss = pssp.tile([P, 1024], f32, name="pss", tag="pss")
                    for j in range(2):
                        ck = 2 * cc + j
                        nc.tensor.matmul(
                            pss[:, j * 512:(j + 1) * 512],
                            KTa[:, h, ck * P:(ck + 1) * P], q_ap,
                            start=True, stop=True)
                    if len(pend) == 2:
                        nc.tensor.matmul(
                            po[:], Va[:, cc - 2, :, h * D:(h + 1) * D],
                            pend.pop(0)[:], start=(cc == 2), stop=False,
                            perf_mode=PM.DoubleRow)
                    e = ep.tile([P, 2, 512], f8, name="e", tag="e")
                    ef = e[:].rearrange("p j q -> p (j q)")
                    nc.scalar.activation(ef, pss[:], FT.Exp,
                                         scale=float(D ** -0.5))
                    if cc == 0:
                        nc.vector.tensor_copy(zacc[:], ef)
                    else:
                        nc.vector.tensor_add(zacc[:], zacc[:], ef)
                    pend.append(e)
                for i, e in enumerate(pend):
                    nc.tensor.matmul(po[:], Va[:, 14 + i, :, h * D:(h + 1) * D],
                                     e[:], start=False, stop=(i == 1),
                                     perf_mode=PM.DoubleRow)
                pz = pozp.tile([P, 512], f32, name="pz", tag="poz")
                for half in range(2):
                    nc.tensor.matmul(pz[:], ones_b[:],
                                     zacc[:, half * 512:(half + 1) * 512],
                                     start=(half == 0), stop=(half == 1))
                invz = zp.tile([P, 512], f32, name="invz", tag="invz")
                zscr = zp.tile([P, 512], f32, name="zscr", tag="zscr")
                nc.vector.reciprocal_approx_accurate(invz[:], pz[:], zscr[:])
                nc.vector.tensor_mul(OTa[:, h, qb * 512:(qb + 1) * 512],
                                     po[:], invz[:])

            def out_chunk(k):
                """out_proj rows [256k, +256) in bf16 + chunked ReduceScatter."""
                for sub in range(2):
                    lc = k * 2 + sub
                    for cb in range(4):
                        pb = pbp.tile([P, 512], f32, name="pb", tag="pb")
                        for h in range(HPC):
                            nc.tensor.matmul(
                                pb[:], OTa[:, h, lc * P:(lc + 1) * P],
                                wo_sb[:, h, cb * 512:(cb + 1) * 512],
                                start=(h == 0), stop=(h == HPC - 1))
                        tb = ob.tile([P, 512], bf16, name="tb", tag="tb")
                        nc.vector.tensor_copy(tb[:], pb[:])
                        nc.sync.dma_start(
                            partial[k][sub * P:(sub + 1) * P,
                                       cb * 512:(cb + 1) * 512], tb[:])
                nc.gpsimd.collective_compute(
                    "ReduceScatter", OP.add,
                    replica_groups=[list(range(NCORES))],
                    ins=[partial[k][:]], outs=[rs_out[k][:]],
                )

            def epilogue(k):
                shb = epi.tile([SH, C], bf16, name="shb", tag="shb")
                nc.sync.dma_start(shb[:], rs_out[k][:])
                posb = epi.tile([SH, C], bf16, name="posb", tag="posb")
                nc.sync.dma_start(posb[:], pos[k * SH:(k + 1) * SH, :])
                shf = epi.tile([SH, C], f32, name="shf", tag="shf")
                nc.vector.tensor_add(shf[:], shb[:], posb[:])
                scr = epi.tile([SH, C], f32, name="scr", tag="scr")
                ssqt = epi.tile([SH, 1], f32, name="ssqt", tag="ssqt")
                nc.scalar.activation(scr[:], shf[:], FT.Square,
                                     accum_out=ssqt[:])
                rmst = epi.tile([SH, 1], f32, name="rmst", tag="rmst")
                nc.scalar.activation(rmst[:], ssqt[:], FT.Sqrt,
                                     bias=eps_t[:SH, 0:1], scale=1.0 / C)
                rinv = epi.tile([SH, 1], f32, name="rinv", tag="rinv")
                nc.vector.reciprocal(rinv[:], rmst[:])
                xt = epi.tile([SH, C], f32, name="xt", tag="xt")
                nc.sync.dma_start(xt[:], xs[k * SH:(k + 1) * SH, :])
                nc.vector.scalar_tensor_tensor(
                    scr[:], shf[:], rinv[:], onw_sb[:SH, :],
                    op0=OP.mult, op1=OP.mult)
                nc.vector.tensor_add(scr[:], scr[:], xt[:])
                nc.sync.dma_start(out[k * SH:(k + 1) * SH, :], scr[:])

            # emission order: out chunks right after both heads of their q
            # block; epilogues trail their RS by >=3 chunks so the Sync queue
            # never head-of-line blocks on a collective
            att(0, 0)
            att(0, 1)
            att(1, 0)
            out_chunk(0)
            out_chunk(1)
            att(1, 1)
            att(2, 0)
            out_chunk(2)
            out_chunk(3)
            att(2, 1)
            att(3, 0)
            out_chunk(4)
            out_chunk(5)
            epilogue(0)
            epilogue(1)
            att(3, 1)
            out_chunk(6)
            out_chunk(7)
            epilogue(2)
            epilogue(3)
            epilogue(4)
            epilogue(5)
            epilogue(6)
            epilogue(7)

    nc.compile()
    return nc


def _rope_mat(depth: float) -> np.ndarray:
    half = D // 2
    freqs = 1.0 / 10000.0 ** (np.arange(half, dtype=np.float32) / half)
    ang = np.float32(depth) * freqs
    c, s = np.cos(ang).astype(np.float32), np.sin(ang).astype(np.float32)
    R = np.zeros((D, D), np.float32)
    R[np.arange(half), np.arange(half)] = c
    R[np.arange(half), np.arange(half) + half] = -s
    R[np.arange(half) + half, np.arange(half)] = s
    R[np.arange(half) + half, np.arange(half) + half] = c
    return R


def _fold_weights(W, norm_w, depth):
    """Per head: R_depth @ diag(norm_w) @ W_head  (rope and norm weight folded)."""
    R = _rope_mat(depth)
    out = np.empty_like(W)
    nheads = W.shape[0] // D
    for h in range(nheads):
        out[h * D:(h + 1) * D] = R @ (norm_w[:, None] * W[h * D:(h + 1) * D])
    return out


def _pack_qk(wf):
    """[CL, C] stationary -> [128, (cc j h m)] fp8 DoubleRow layout."""
    wt = np.ascontiguousarray(wf.T)              # [C, CL]
    wt = wt.reshape(8, 2, P, HPC, D).transpose(2, 0, 1, 3, 4)
    return np.ascontiguousarray(wt.reshape(P, 4096)).astype(F8)


def _pack_v(wf):
    """[CL, C] moving -> [128, (cc j m)] fp8 DoubleRow layout."""
    wt = np.ascontiguousarray(wf.T)              # [C, CL]
    wt = wt.reshape(8, 2, P, CL).transpose(2, 0, 1, 3)
    return np.ascontiguousarray(wt.reshape(P, 4096)).astype(F8)


def _pack_h(h2d):
    """h [L, C] -> strip layout [p, (q4 cc j col)] fp8."""
    hp = h2d.reshape(4, 512, 8, 2, P).transpose(4, 0, 2, 3, 1)
    return np.ascontiguousarray(hp.reshape(P, 4 * 8192)).astype(F8)


def kernel(**inputs) -> np.ndarray:
    inputs = {k: np.asarray(v, dtype=np.float32) if np.asarray(v).dtype != np.int32
              else np.asarray(v) for k, v in inputs.items()}
    x = inputs["x"]
    qn, kn = inputs["qn_w"], inputs["kn_w"]

    # rmsnorm scale is computed on-device from the roped/weighted projection;
    # exact when qn_w/kn_w are all ones (rope is orthogonal).
    if not (np.allclose(qn, 1.0) and np.allclose(kn, 1.0)):
        raise NotImplementedError("non-unit q/k norm weights not supported")

    if "prog" not in _CACHE:
        _CACHE["prog"] = _build_program()
    nc = _CACHE["prog"]

    hT8 = [_pack_h(inputs[f"h{t}"][0]) for t in range(3)]

    # SIREN positional field is input-independent: fold on host (x VS to match
    # the on-device scale; the final rmsnorm cancels it).
    coords = np.linspace(-1.0, 1.0, L, dtype=np.float32)[:, None]
    posf = (np.sin(30.0 * (coords @ inputs["sw1"] + inputs["sb1"][None, :]))
            @ inputs["sw2"] + inputs["sb2"][None, :]) * np.float32(VS)

    in_maps = []
    for i in range(NCORES):
        sl = slice(i * CL, (i + 1) * CL)
        wq_f = _fold_weights(inputs["Wq"][sl], qn, 2.0) * np.float32(WS)
        wk0_f = _fold_weights(inputs["Wk"][sl], kn, 0.0) * np.float32(WS)
        wk1_f = _fold_weights(inputs["Wk"][sl], kn, 1.0) * np.float32(WS)
        wv_f = inputs["Wv"][sl] * np.float32(VS)
        rows = np.concatenate(
            [np.arange(k * 256 + i * SH, k * 256 + (i + 1) * SH)
             for k in range(NCH)])
        in_maps.append({
            "hTp0": hT8[0], "hTp1": hT8[1], "hTp2": hT8[2],
            "wq": _pack_qk(wq_f),
            "wk0": _pack_qk(wk0_f),
            "wk1": _pack_qk(wk1_f),
            "wv": _pack_v(wv_f),
            "wo": np.ascontiguousarray(inputs["Wo"][:, sl].T).astype(BF),
            "onw": np.ascontiguousarray(
                np.broadcast_to(inputs["on_w"][None, :], (P, C))),
            "xs": np.ascontiguousarray(x[0, rows, :]),
            "pos": np.ascontiguousarray(posf[rows, :]).astype(BF),
        })

    _CACHE["last_in_maps"] = in_maps
    res = run_bass_kernel_spmd(nc, in_maps, list(range(NCORES)))
    out = np.empty((1, L, C), np.float32)
    for i in range(NCORES):
        o = res.results[i]["o"]
        for k in range(NCH):
            out[0, k * 256 + i * SH:k * 256 + (i + 1) * SH, :] = \
                o[k * SH:(k + 1) * SH, :]
    return out


# revision 15
# speedup vs baseline: 1.1537x; 1.1537x over previous
"""TRN2 Bass kernel for nn_CrossLayerAttention: head-parallel tensor-parallel
over 8 NeuronCores.

v3: PE-continuity-focused schedule. The TRN2 PE clock ramps (0.65/1.2/2.4 GHz)
with sustained execution and back-to-back matmuls issue every ~216ns, so the
kernel is organized to keep the tensor-engine queue free of dependency waits:

  - host pre-permutes h.T into per-sweep strip layout: one 1 MB DMA per
    projection sweep (no per-chunk DMA issue pressure on the Sync queue)
  - per sweep: all K matmuls (fp8 DoubleRow, 256-deep), then all V matmuls
    (natural layout, strip-stationary), then the rms ones-matmul into a PSUM
    slot recycled from the drained K accumulator (kps/ssq share a 2-ring)
  - rmsnorm chain entirely off-PE: DVE drain + square, PE ones-matmul, Scalar
    sqrt, DVE reciprocal_approx_accurate, DVE scale
  - attention: QK bf16; exp over [128,1024] PSUM pairs -> fp8 e pairs; AV
    (fp8 DoubleRow) emitted TWO chunks behind QK so the PE never waits on the
    exp stream; Z accumulated bf16 on DVE + ones-matmul
  - out_proj bf16, PSUM->bf16 drains on DVE; partials bf16; 8 ReduceScatter
    chunks overlapped with attention; dummy warmup collective at t=0;
    epilogues scheduled >=3 chunks behind their RS
  - SIREN positional field computed on host (input-independent), added in the
    epilogue before the final rmsnorm (scale-invariance folds the fp8 scales)
"""
import numpy as np
import ml_dtypes
from contextlib import ExitStack

import concourse.bass as bass
import concourse.tile as tile
from concourse import bacc, mybir
from concourse.bass_utils import run_bass_kernel_spmd

P = 128
L = 2048
C = 2048
H = 16
D = 128
NCORES = 8
HPC = H // NCORES          # heads per core
CL = HPC * D               # local channels per core
LKV = 2 * L                # kv length (2 history entries)
EPS = 1e-6
NQB = 4                    # attention q blocks (512 wide)
NCH = 4                    # out_proj / RS chunks (512 rows)
SH = L // 8 // NCH         # shard rows per RS chunk (64)
WS = 32.0                  # wq/wk fp8 scale (cancels in q/k rmsnorm)
VS = 16.0                  # wv fp8 scale (cancels in final rmsnorm)

f32 = mybir.dt.float32
bf16 = mybir.dt.bfloat16
f8 = mybir.dt.float8e4
FT = mybir.ActivationFunctionType
OP = mybir.AluOpType
PM = mybir.MatmulPerfMode
BF = ml_dtypes.bfloat16
F8 = ml_dtypes.float8_e4m3

_CACHE = {}


def _build_program():
    nc = bacc.Bacc("TRN2", target_bir_lowering=False, debug=False,
                   num_devices=NCORES)

    # ---- DRAM I/O ----
    # hTp[t]: pre-permuted strips, [p, (q4, cc, j, col)]
    hTp = [nc.dram_tensor(f"hTp{t}", [P, 4 * 8192], f8, kind="ExternalInput")
           for t in range(3)]
    wq = nc.dram_tensor("wq", [P, 4096], f8, kind="ExternalInput")
    wk0 = nc.dram_tensor("wk0", [P, 4096], f8, kind="ExternalInput")
    wk1 = nc.dram_tensor("wk1", [P, 4096], f8, kind="ExternalInput")
    wv = nc.dram_tensor("wv", [P, 4096], f8, kind="ExternalInput")
    wo = nc.dram_tensor("wo", [CL, C], bf16, kind="ExternalInput")
    onw = nc.dram_tensor("onw", [P, C], f32, kind="ExternalInput")
    xs = nc.dram_tensor("xs", [NCH * SH, C], f32, kind="ExternalInput")
    pos = nc.dram_tensor("pos", [NCH * SH, C], bf16, kind="ExternalInput")
    out = nc.dram_tensor("o", [NCH * SH, C], f32, kind="ExternalOutput")

    partial = [nc.dram_tensor(f"partial{k}", [512, C], bf16)
               for k in range(NCH)]
    rs_out = [nc.dram_tensor(f"rs_out{k}", [SH, C], bf16) for k in range(NCH)]
    wdum = nc.dram_tensor("wdum", [8, 8], f32)
    wrs = nc.dram_tensor("wrs", [1, 8], f32)

    with tile.TileContext(nc) as tc, ExitStack() as ctx:
        const = ctx.enter_context(tc.tile_pool(name="const", bufs=1))
        persist = ctx.enter_context(tc.tile_pool(name="persist", bufs=1))

        # ---- constants ----
        ones_t = const.tile([P, P], f32)
        nc.vector.memset(ones_t[:], 1.0)
        ones_b = const.tile([P, P], bf16)
        nc.vector.tensor_copy(ones_b[:], ones_t[:])
        eps_t = const.tile([P, 1], f32)
        nc.vector.memset(eps_t[:], EPS)
        ones8 = const.tile([P, 2, P], f8)
        nc.vector.memset(ones8[:], 1.0)

        # ---- warmup collective (absorb ring setup during projections) ----
        wdum_sb = const.tile([8, 8], f32)
        nc.vector.memset(wdum_sb[:], 0.0)
        nc.sync.dma_start(wdum[:], wdum_sb[:])
        nc.gpsimd.collective_compute(
            "ReduceScatter", OP.add,
            replica_groups=[list(range(NCORES))],
            ins=[wdum[:]], outs=[wrs[:]],
        )

        # ---- persistent activations / weights ----
        QTa = persist.tile([P, HPC, L], bf16, name="QTa")
        KTa = persist.tile([P, HPC, LKV], bf16, name="KTa")
        Va = persist.tile([P, LKV // 256, 2, CL], f8, name="Va")
        OTa = persist.tile([P, HPC, L], bf16, name="OTa")
        wo_sb = persist.tile([P, HPC, C], bf16, name="wo_sb")
        onw_sb = persist.tile([P, C], f32, name="onw_sb")

        # ================= projections =================
        wp_cm = tc.tile_pool(name="wp", bufs=1)
        wp = wp_cm.__enter__()
        wq_sb = wp.tile([P, 8, 2, HPC, D], f8, name="wq_sb")
        wk_sb = [wp.tile([P, 8, 2, HPC, D], f8, name=f"wk{t}_sb")
                 for t in range(2)]
        wv_sb = wp.tile([P, 8, 2, CL], f8, name="wv_sb")
        # weight DMAs needed by the first sweeps go first
        nc.sync.dma_start(
            wk_sb[0][:].rearrange("p a j h m -> p (a j h m)"), wk0[:])
        nc.sync.dma_start(wv_sb[:].rearrange("p a j m -> p (a j m)"), wv[:])

        pp_cm = tc.tile_pool(name="pp", bufs=1, space="PSUM")
        pp = pp_cm.__enter__()
        hp_cm = tc.tile_pool(name="hp", bufs=4)
        hp = hp_cm.__enter__()
        rp_cm = tc.tile_pool(name="rp", bufs=2)
        rp = rp_cm.__enter__()

        def sweep(t, q4, w_sb, dst_tile, dst_off, with_v):
            """Project strips of hT[t], cols [512*q4, +512): K (+V), 2 heads."""
            strip = hp.tile([P, 8, 2, 512], f8, name="strip", tag="strip")
            nc.sync.dma_start(
                strip[:].rearrange("p a j q -> p (a j q)"),
                hTp[t][:, q4 * 8192:(q4 + 1) * 8192])
            kps = pp.tile([P, 1024], f32, name="kps", tag="kps", bufs=2)
            # K: all 16 matmuls back-to-back (strip prefetched, no deps)
            for cc in range(8):
                for h in range(HPC):
                    nc.tensor.matmul(
                        kps[:, h * 512:(h + 1) * 512],
                        w_sb[:, cc, :, h, :], strip[:, cc, :, :],
                        start=(cc == 0), stop=(cc == 7),
                        perf_mode=PM.DoubleRow)
            # rms drain chain starts (off-PE) while V matmuls run
            raw = rp.tile([P, 1024], bf16, name="raw", tag="raw")
            nc.vector.tensor_copy(raw[:], kps[:])
            sq = rp.tile([P, 1024], bf16, name="sq", tag="sq")
            nc.vector.tensor_mul(sq[:], raw[:], raw[:])
            if with_v:
                vps = [pp.tile([P, 256], f32, name=f"vb{lb}", tag=f"vb{lb}",
                               bufs=1) for lb in range(4)]
                for cc in range(8):
                    for lb in range(4):
                        nc.tensor.matmul(
                            vps[lb][:],
                            strip[:, cc, :, lb * P:(lb + 1) * P],
                            wv_sb[:, cc, :, :],
                            start=(cc == 0), stop=(cc == 7),
                            perf_mode=PM.DoubleRow)
                for lb in range(4):
                    ck = t * 16 + q4 * 4 + lb
                    nc.vector.tensor_copy(Va[:, ck // 2, ck % 2, :],
                                          vps[lb][:])
            # partition-sum of squares into the PSUM slot the K drain freed
            ssq = pp.tile([P, 1024], f32, name="ssq", tag="kps", bufs=2)
            for half in range(2):
                nc.tensor.matmul(ssq[:, half * 512:(half + 1) * 512],
                                 ones_b[:], sq[:, half * 512:(half + 1) * 512],
                                 start=True, stop=True)
            rms = rp.tile([P, 1024], f32, name="rms", tag="rms")
            nc.scalar.activation(rms[:], ssq[:], FT.Sqrt,
                                 bias=eps_t[:, 0:1], scale=1.0 / D)
            inv = rp.tile([P, 1024], f32, name="inv", tag="inv")
            scr8 = rp.tile([P, 1024], f32, name="scr8", tag="scr8")
            nc.vector.reciprocal_approx_accurate(inv[:], rms[:], scr8[:])
            nc.vector.tensor_mul(
                dst_tile[:, :, dst_off:dst_off + 512],
                raw[:].rearrange("p (h q) -> p h q", h=2),
                inv[:].rearrange("p (h q) -> p h q", h=2))

        sweep(0, 0, wk_sb[0], KTa, 0, True)
        # remaining weight loads, issued behind the first sweep's strip DMA
        nc.sync.dma_start(
            wk_sb[1][:].rearrange("p a j h m -> p (a j h m)"), wk1[:])
        nc.sync.dma_start(wq_sb[:].rearrange("p a j h m -> p (a j h m)"),
                          wq[:])
        for h in range(HPC):
            nc.sync.dma_start(wo_sb[:, h, :], wo[h * P:(h + 1) * P, :])
        nc.sync.dma_start(onw_sb[:], onw[:])
        for q4 in range(1, 4):
            sweep(0, q4, wk_sb[0], KTa, q4 * 512, True)
        for q4 in range(4):
            sweep(1, q4, wk_sb[1], KTa, L + q4 * 512, True)
        for q4 in range(4):
            sweep(2, q4, wq_sb, QTa, q4 * 512, False)

        rp_cm.__exit__(None, None, None)
        hp_cm.__exit__(None, None, None)
        pp_cm.__exit__(None, None, None)
        wp_cm.__exit__(None, None, None)

        # ===== attention / out_proj / RS / epilogue, PE-order interleaved ====
        with (tc.tile_pool(name="pssp", bufs=2, space="PSUM") as pssp,
              tc.tile_pool(name="pop", bufs=1, space="PSUM") as pop,
              tc.tile_pool(name="pzp", bufs=1, space="PSUM") as pzp,
              tc.tile_pool(name="pbp", bufs=2, space="PSUM") as pbp,
              tc.tile_pool(name="ep", bufs=5) as ep,
              tc.tile_pool(name="zp", bufs=2) as zp,
              tc.tile_pool(name="ob", bufs=3) as ob,
              tc.tile_pool(name="epi", bufs=2) as epi):

            def att(qb, h):
                po = pop.tile([P, 512], f32, name="po", tag="po")
                pz = pzp.tile([P, 512], f32, name="pz", tag="pz")
                q_ap = QTa[:, h, qb * 512:(qb + 1) * 512]
                pend = []  # AV/Z emission lags QK by 2 chunks: PE never waits
                for cc in range(16):
                    pss = pssp.tile([P, 1024], f32, name="pss", tag="pss")
                    for j in range(2):
                        ck = 2 * cc + j
                        nc.tensor.matmul(
                            pss[:, j * 512:(j + 1) * 512],
                            KTa[:, h, ck * P:(ck + 1) * P], q_ap,
                            start=True, stop=True)
                    if len(pend) == 2:
                        ep_ = pend.pop(0)
                        nc.tensor.matmul(
                            po[:], Va[:, cc - 2, :, h * D:(h + 1) * D],
                            ep_[:], start=(cc == 2), stop=False,
                            perf_mode=PM.DoubleRow)
                        nc.tensor.matmul(
                            pz[:], ones8[:], ep_[:], start=(cc == 2),
                            stop=False, perf_mode=PM.DoubleRow)
                    e = ep.tile([P, 2, 512], f8, name="e", tag="e")
                    nc.scalar.activation(e[:].rearrange("p j q -> p (j q)"),
                                         pss[:], FT.Exp,
                                         scale=float(D ** -0.5))
                    pend.append(e)
                for i, e in enumerate(pend):
                    nc.tensor.matmul(po[:], Va[:, 14 + i, :, h * D:(h + 1) * D],
                                     e[:], start=False, stop=(i == 1),
                                     perf_mode=PM.DoubleRow)
                    nc.tensor.matmul(pz[:], ones8[:], e[:], start=False,
                                     stop=(i == 1), perf_mode=PM.DoubleRow)
                invz = zp.tile([P, 512], f32, name="invz", tag="invz")
                zscr = zp.tile([P, 512], f32, name="zscr", tag="zscr")
                nc.vector.reciprocal_approx_accurate(invz[:], pz[:], zscr[:])
                nc.vector.tensor_mul(OTa[:, h, qb * 512:(qb + 1) * 512],
                                     po[:], invz[:])

            def out_chunk(k):
                """out_proj rows [512k, +512) in bf16 + chunked ReduceScatter."""
                for sub in range(4):
                    lc = k * 4 + sub
                    for cb in range(4):
                        pb = pbp.tile([P, 512], f32, name="pb", tag="pb")
                        for h in range(HPC):
                            nc.tensor.matmul(
                                pb[:], OTa[:, h, lc * P:(lc + 1) * P],
                                wo_sb[:, h, cb * 512:(cb + 1) * 512],
                                start=(h == 0), stop=(h == HPC - 1))
                        tb = ob.tile([P, 512], bf16, name="tb", tag="tb")
                        nc.vector.tensor_copy(tb[:], pb[:])
                        nc.sync.dma_start(
                            partial[k][sub * P:(sub + 1) * P,
                                       cb * 512:(cb + 1) * 512], tb[:])
                nc.gpsimd.collective_compute(
                    "ReduceScatter", OP.add,
                    replica_groups=[list(range(NCORES))],
                    ins=[partial[k][:]], outs=[rs_out[k][:]],
                )

            def epilogue(k):
                shb = epi.tile([SH, C], bf16, name="shb", tag="shb")
                nc.sync.dma_start(shb[:], rs_out[k][:])
                posb = epi.tile([SH, C], bf16, name="posb", tag="posb")
                nc.sync.dma_start(posb[:], pos[k * SH:(k + 1) * SH, :])
                shf = epi.tile([SH, C], f32, name="shf", tag="shf")
                nc.vector.tensor_add(shf[:], shb[:], posb[:])
                scr = epi.tile([SH, C], f32, name="scr", tag="scr")
                ssqt = epi.tile([SH, 1], f32, name="ssqt", tag="ssqt")
                nc.scalar.activation(scr[:], shf[:], FT.Square,
                                     accum_out=ssqt[:])
                rmst = epi.tile([SH, 1], f32, name="rmst", tag="rmst")
                nc.scalar.activation(rmst[:], ssqt[:], FT.Sqrt,
                                     bias=eps_t[:SH, 0:1], scale=1.0 / C)
                rinv = epi.tile([SH, 1], f32, name="rinv", tag="rinv")
                nc.vector.reciprocal(rinv[:], rmst[:])
                xt = epi.tile([SH, C], f32, name="xt", tag="xt")
                nc.sync.dma_start(xt[:], xs[k * SH:(k + 1) * SH, :])
                nc.vector.scalar_tensor_tensor(
                    scr[:], shf[:], rinv[:], onw_sb[:SH, :],
                    op0=OP.mult, op1=OP.mult)
                nc.vector.tensor_add(scr[:], scr[:], xt[:])
                nc.sync.dma_start(out[k * SH:(k + 1) * SH, :], scr[:])

            # emission order: out chunk right after both heads of its q block;
            # epilogues trail their RS by >=2 chunks so the Sync queue never
            # head-of-line blocks on a collective
            att(0, 0)
            att(0, 1)
            out_chunk(0)
            att(1, 0)
            att(1, 1)
            out_chunk(1)
            att(2, 0)
            att(2, 1)
            out_chunk(2)
            epilogue(0)
            att(3, 0)
            att(3, 1)
            out_chunk(3)
            epilogue(1)
            epilogue(2)
            epilogue(3)

    nc.compile()
    return nc


def _rope_mat(depth: float) -> np.ndarray:
    half = D // 2
    freqs = 1.0 / 10000.0 ** (np.arange(half, dtype=np.float32) / half)
    ang = np.float32(depth) * freqs
    c, s = np.cos(ang).astype(np.float32), np.sin(ang).astype(np.float32)
    R = np.zeros((D, D), np.float32)
    R[np.arange(half), np.arange(half)] = c
    R[np.arange(half), np.arange(half) + half] = -s
    R[np.arange(half) + half, np.arange(half)] = s
    R[np.arange(half) + half, np.arange(half) + half] = c
    return R


def _fold_weights(W, norm_w, depth):
    """Per head: R_depth @ diag(norm_w) @ W_head  (rope and norm weight folded)."""
    R = _rope_mat(depth)
    out = np.empty_like(W)
    nheads = W.shape[0] // D
    for h in range(nheads):
        out[h * D:(h + 1) * D] = R @ (norm_w[:, None] * W[h * D:(h + 1) * D])
    return out


def _pack_qk(wf):
    """[CL, C] stationary -> [128, (cc j h m)] fp8 DoubleRow layout."""
    wt = np.ascontiguousarray(wf.T)              # [C, CL]
    wt = wt.reshape(8, 2, P, HPC, D).transpose(2, 0, 1, 3, 4)
    return np.ascontiguousarray(wt.reshape(P, 4096)).astype(F8)


def _pack_v(wf):
    """[CL, C] moving -> [128, (cc j m)] fp8 DoubleRow layout."""
    wt = np.ascontiguousarray(wf.T)              # [C, CL]
    wt = wt.reshape(8, 2, P, CL).transpose(2, 0, 1, 3)
    return np.ascontiguousarray(wt.reshape(P, 4096)).astype(F8)


def _pack_h(h2d):
    """h [L, C] -> strip layout [p, (q4 cc j col)] fp8."""
    hp = h2d.reshape(4, 512, 8, 2, P).transpose(4, 0, 2, 3, 1)
    return np.ascontiguousarray(hp.reshape(P, 4 * 8192)).astype(F8)


def kernel(**inputs) -> np.ndarray:
    inputs = {k: np.asarray(v, dtype=np.float32) if np.asarray(v).dtype != np.int32
              else np.asarray(v) for k, v in inputs.items()}
    x = inputs["x"]
    qn, kn = inputs["qn_w"], inputs["kn_w"]

    # rmsnorm scale is computed on-device from the roped/weighted projection;
    # exact when qn_w/kn_w are all ones (rope is orthogonal).
    if not (np.allclose(qn, 1.0) and np.allclose(kn, 1.0)):
        raise NotImplementedError("non-unit q/k norm weights not supported")

    if "prog" not in _CACHE:
        _CACHE["prog"] = _build_program()
    nc = _CACHE["prog"]

    hT8 = [_pack_h(inputs[f"h{t}"][0]) for t in range(3)]

    # SIREN positional field is input-independent: fold on host (x VS to match
    # the on-device scale; the final rmsnorm cancels it).
    coords = np.linspace(-1.0, 1.0, L, dtype=np.float32)[:, None]
    posf = (np.sin(30.0 * (coords @ inputs["sw1"] + inputs["sb1"][None, :]))
            @ inputs["sw2"] + inputs["sb2"][None, :]) * np.float32(VS)

    in_maps = []
    for i in range(NCORES):
        sl = slice(i * CL, (i + 1) * CL)
        wq_f = _fold_weights(inputs["Wq"][sl], qn, 2.0) * np.float32(WS)
        wk0_f = _fold_weights(inputs["Wk"][sl], kn, 0.0) * np.float32(WS)
        wk1_f = _fold_weights(inputs["Wk"][sl], kn, 1.0) * np.float32(WS)
        wv_f = inputs["Wv"][sl] * np.float32(VS)
        rows = np.concatenate(
            [np.arange(k * 512 + i * SH, k * 512 + (i + 1) * SH)
             for k in range(NCH)])
        in_maps.append({
            "hTp0": hT8[0], "hTp1": hT8[1], "hTp2": hT8[2],
            "wq": _pack_qk(wq_f),
            "wk0": _pack_qk(wk0_f),
            "wk1": _pack_qk(wk1_f),
            "wv": _pack_v(wv_f),
            "wo": np.ascontiguousarray(inputs["Wo"][:, sl].T).astype(BF),
            "onw": np.ascontiguousarray(
                np.broadcast_to(inputs["on_w"][None, :], (P, C))),
            "xs": np.ascontiguousarray(x[0, rows, :]),
            "pos": np.ascontiguousarray(posf[rows, :]).astype(BF),
        })

    _CACHE["last_in_maps"] = in_maps
    res = run_bass_kernel_spmd(nc, in_maps, list(range(NCORES)))
    out = np.empty((1, L, C), np.float32)
    for i in range(NCORES):
        o = res.results[i]["o"]
        for k in range(NCH):
            out[0, k * 512 + i * SH:k * 512 + (i + 1) * SH, :] = \
                o[k * SH:(k + 1) * SH, :]
    return out
